# revision 13
# baseline (speedup 1.0000x reference)
"""Trainium2 Bass kernel for nn_ComputeCorr (retrieval_knn).

Math (per batch pair b, D=64 features):
  d[n,m] = ||sf[n]-tf[m]||^2,  sf = src_f[b].T, tf = tgt_f[b].T
  src_corr[b] = softmax_m(-d) @ tgt[b];  tgt_corr[b] = softmax_n(-d.T) @ src[b]

Restructure (per side, shown for src_corr):
  softmax_m(-d)[n,:] @ tgt = (sum_m U[m,n] * [tgt|1][m,:])[:3] / (...)[3]
  U[m,n] = exp(c0 - d[n,m]) computed directly in [m(part), n(free)] layout.
  The whole exponent (2*ab - aa[n] - bb[m] + c0)/2 comes from ONE fp16
  matmul with K-packed augmentation rows (K = 64 + 3):
    lhsT = [fp16(L); 1; bias_hi; bias_lo]   (bias = (c0 - |l_m|^2)/2)
    rhs  = [fp16(R); -|r_n|^2/2; 1; 1]
  so exponent == c0 - d <= c0: no max pass, no overflow, and the fp16
  rounding of the -|r_n|^2/2 row is a per-column shift that cancels in the
  softmax normalization.

  exp is split across TWO engines (the single-ScalarE exp cadence was the
  original steady-state limiter): even groups use ScalarE's native Exp
  (bf16 out); odd groups use a one-instruction Schraudolph bit-trick on
  the DVE: u16 = sat_round(s*2*128/ln2 + 16248.67), a fp32->uint16
  converting tensor_scalar whose output bits ARE the bf16 weight
  (negative/underflowed exponents saturate to 0). The u tile is allocated
  bf16 and written through a uint16 bitcast view so the consuming matmul
  reads a plain bf16 tile.

  The PE stream is software-pipelined with lag 2: scores of group g+2 are
  emitted before the PV matmuls of group g, so the exp/schrau latency of
  group g is hidden under 3 groups of PE work instead of stalling the PV.
  PSUM: 3 x s[128,1024] (6 banks) + 2 x w[128,512] (2 banks) = 8 banks;
  the w pool is double-buffered so the pipeline flows across n-block
  boundaries.

  Outputs are written as [3, H] (transposed; host transposes back): the
  epilogue is then reciprocal + 3 row multiplies ([1,512] each, moved off
  the critical engines to GpSimd) + a 3-descriptor contiguous DMA - no PE
  transpose, no PSUM transpose bank.

Sharding: 8 cores = 4 batches x 2 halves; core c = batch c//2, rows
[h*2048,(h+1)*2048) of BOTH outputs (h=c%2). The [4096 x 2048] score
block per side is never materialized in DRAM.

DMA: all inputs ride the sync HWDGE ring in exact consumption order
(side A first); output stores interleave behind them. The tiny v tensors
go on the gpsimd SWDGE ring.
"""

import os
import sys

import numpy as np

for _p in ("/opt/trn_rl_repo", "/root/.axon_site/_ro/trn_rl_repo"):
    if os.path.isdir(_p) and _p not in sys.path:
        sys.path.insert(0, _p)

import ml_dtypes

import concourse.bacc as bacc
import concourse.tile as tile
from concourse import mybir
from concourse.bass_utils import run_bass_kernel_spmd

B, N, M, D = 4, 4096, 4096, 64
H = N // 2  # rows per core per side
NCORES = 8
C0 = 40.0
KS = D + 3  # score matmul contraction: features + shift row + 2 bias rows
MB = 128  # m block (score partition dim)
NB = 512  # matmul free dim (PSUM bank)
NMB = M // MB  # 32 m blocks
NNB = H // NB  # 4 n blocks per core
VW = 256  # padded width of the v tensor (128-wide lhsT slices)
F32 = mybir.dt.float32
F16 = mybir.dt.float16
U16 = mybir.dt.uint16
BF16 = mybir.dt.bfloat16
NPBF = ml_dtypes.bfloat16

# Schraudolph bf16-bit-trick exp constants: for score s (= exponent/2),
# bf16_bits ~= s * (2*128/ln2) + (127<<7) - 7.33 (sawtooth-centering shift)
SCH_A = 2.0 * 128.0 / float(np.log(2.0))
SCH_B = 16256.0 - 128.0 * 0.0573

LAG = 3  # PV lags the score/exp stream by this many groups

_PROG = None


# lhs DMA column chunks; smaller leading chunks unblock the first matmuls
# sooner (the sync ring's descriptor rate is the startup bottleneck)
LHS_CHUNKS = (512, 1024, 1024, 1536)


def _alloc_side(big, names):
    side = names["side"]
    lhs_ch = [
        big.tile([KS, w], F16, tag=f"lhs{i}{side}", name=f"lhs{i}")
        for i, w in enumerate(LHS_CHUNKS)
    ]
    rhs = big.tile([KS, H], F16, tag=f"rhs{side}", name="rhs")
    v_sb = big.tile([MB, VW], F16, tag=f"v{side}", name="v")

    bounds = []
    b0 = 0
    for w in LHS_CHUNKS:
        bounds.append((b0, b0 + w))
        b0 += w

    def lhs_slice(mi):
        col = mi * MB
        for c, (lo, hi) in enumerate(bounds):
            if col < hi:
                return lhs_ch[c][:, col - lo : col - lo + MB]
        raise AssertionError

    return lhs_ch, rhs, v_sb, lhs_slice


def _load_sides(nc, big, all_names):
    """Inputs ride the sync HWDGE ring in consumption order, with side B's
    first pieces inserted into side A's slack so the side transition never
    stalls. The first rhs bank is split by K-rows across the sync and
    scalar HWDGE rings (the scalar ring is otherwise idle and its one tiny
    trigger at t=0 can't delay ScalarE's first exp). v rides SWDGE."""
    nmA, nmB = all_names
    lhsA, rhsA, vA, sliceA = _alloc_side(big, nmA)
    lhsB, rhsB, vB, sliceB = _alloc_side(big, nmB)
    nc.gpsimd.dma_start(out=vA, in_=nmA["v"])
    nc.gpsimd.dma_start(out=vB, in_=nmB["v"])

    KH = KS // 2
    bounds = np.cumsum((0,) + LHS_CHUNKS)

    def lhs_chunk(nc_ring, lhs_ch, names, i):
        nc_ring.dma_start(
            out=lhs_ch[i], in_=names["lhs"][:, bounds[i] : bounds[i + 1]]
        )

    # Critical path to the first matmul: lhsA chunk 0 + rhsA bank 0 (split
    # by K-rows across the sync and scalar rings). The sync ring then
    # carries only lhs chunks + rhsB bank 0, pacing n-block 0's appetite;
    # the rhs tails (n-blocks 1-3, not needed for ~14us per side) ride the
    # gpsimd SWDGE ring, whose transfers process on separate DMA engines.
    nc.scalar.dma_start(out=rhsA[KH:, :NB], in_=nmA["rhs"][KH:, :NB])
    lhs_chunk(nc.sync, lhsA, nmA, 0)
    nc.sync.dma_start(out=rhsA[:KH, :NB], in_=nmA["rhs"][:KH, :NB])
    nc.gpsimd.dma_start(out=rhsA[:, NB:], in_=nmA["rhs"][:, NB:])
    for i in range(1, len(LHS_CHUNKS)):
        lhs_chunk(nc.sync, lhsA, nmA, i)
    # side B: ready long before the side transition
    lhs_chunk(nc.sync, lhsB, nmB, 0)
    nc.sync.dma_start(out=rhsB[:, :NB], in_=nmB["rhs"][:, :NB])
    nc.gpsimd.dma_start(out=rhsB[:, NB:], in_=nmB["rhs"][:, NB:])
    for i in range(1, len(LHS_CHUNKS)):
        lhs_chunk(nc.sync, lhsB, nmB, i)
    return (sliceA, rhsA, vA), (sliceB, rhsB, vB)


def _build_side(nc, pools, loaded, out_d, ring):
    big, upool, spool, wpool = pools
    lhs_slice, rhs, v_sb = loaded

    def emit_scores(nj, gi):
        ncol = slice(nj * NB, (nj + 1) * NB)
        s = spool.tile([MB, 2 * NB], F32, tag="s", name="s")
        for half in range(2):
            mi = gi * 2 + half
            nc.tensor.matmul(
                s[:, half * NB : (half + 1) * NB],
                lhsT=lhs_slice(mi),
                rhs=rhs[:, ncol],
                start=True,
                stop=True,
            )
        if gi % 2 == 0:
            u = upool.tile([MB, 2 * NB], BF16, tag="u", name="u")
            nc.scalar.activation(
                out=u, in_=s, func=mybir.ActivationFunctionType.Exp, scale=2.0
            )
        else:
            # Schraudolph on the DVE: uint16-saturating affine of the score
            # IS the bf16 weight bit pattern (written via a bitcast view so
            # the consumer reads a plain bf16 tile).
            u = upool.tile([MB, 2 * NB], BF16, tag="u16", name="u16")
            nc.vector.tensor_scalar(
                u[:, :].bitcast(U16),
                s,
                SCH_A,
                SCH_B,
                mybir.AluOpType.mult,
                mybir.AluOpType.add,
            )
        return u

    def emit_pv(w, gi, u):
        for half in range(2):
            mi = gi * 2 + half
            nc.tensor.matmul(
                w,
                lhsT=v_sb[:, mi * 4 : mi * 4 + MB],
                rhs=u[:, half * NB : (half + 1) * NB],
                start=(mi == 0),
                stop=(mi == NMB - 1),
            )

    def emit_epilogue(w, nj):
        # w rows 0..2 are the numerator, row 3 the denominator. Copy the
        # [4, NB] block to SBUF (frees the PSUM bank) and store it raw; the
        # host does the tiny divide during unsharding.
        w_sb = upool.tile([4, NB], F32, tag="wsb", name="wsb")
        nc.scalar.copy(w_sb, w[0:4, :])
        ring.dma_start(out=out_d[:, nj * NB : (nj + 1) * NB], in_=w_sb)

    NG = NMB // 2  # score/exp groups per n-block
    EPLAG = 3  # units between a w-block's last PV and its epilogue emission
    units = [(nj, gi) for nj in range(NNB) for gi in range(NG)]
    fifo = []
    epi_fifo = []
    w_cur = None
    for idx in range(len(units) + LAG):
        if idx < len(units):
            nj, gi = units[idx]
            u = emit_scores(nj, gi)
            fifo.append((nj, gi, u))
        if epi_fifo and epi_fifo[0][0] <= idx:
            emit_epilogue(*epi_fifo.pop(0)[1:])
        if idx >= LAG:
            nj, gi, u = fifo.pop(0)
            if gi == 0:
                w_cur = wpool.tile([MB, NB], F32, tag="w", name="w")
            emit_pv(w_cur, gi, u)
            if gi == NG - 1:
                # defer the PSUM->SBUF copy + store so the in-order engine
                # doing the copy never stalls waiting for the last PV
                epi_fifo.append((idx + EPLAG, w_cur, nj))
    for _, w, nj in epi_fifo:
        emit_epilogue(w, nj)


def _build():
    nc = bacc.Bacc("TRN2", target_bir_lowering=False, debug=False)

    sides = []
    for side in ("A", "B"):
        sides.append(
            {
                "side": side,
                "lhs": nc.dram_tensor(
                    f"lhs{side}", [KS, M], F16, kind="ExternalInput"
                ).ap(),
                "rhs": nc.dram_tensor(
                    f"rhs{side}", [KS, H], F16, kind="ExternalInput"
                ).ap(),
                "v": nc.dram_tensor(
                    f"v{side}", [MB, VW], F16, kind="ExternalInput"
                ).ap(),
            }
        )
    out_src = nc.dram_tensor("out_src", [4, H], F32, kind="ExternalOutput").ap()
    out_tgt = nc.dram_tensor("out_tgt", [4, H], F32, kind="ExternalOutput").ap()

    with tile.TileContext(nc) as tc:
        with (
            tc.tile_pool(name="big", bufs=2) as big,
            tc.tile_pool(name="upool", bufs=6) as upool,
            tc.tile_pool(name="spool", bufs=3, space="PSUM") as spool,
            tc.tile_pool(name="wpool", bufs=2, space="PSUM") as wpool,
        ):
            pools = (big, upool, spool, wpool)
            ldA, ldB = _load_sides(nc, big, sides)
            _build_side(nc, pools, ldA, out_src, nc.sync)
            _build_side(nc, pools, ldB, out_tgt, nc.sync)

    nc.compile()
    return nc


def _hi_lo16(x):
    hi = x.astype(np.float16)
    lo = (x - hi.astype(np.float32)).astype(np.float16)
    return hi, lo


def _prep_inputs(src, tgt, src_f, tgt_f):
    """Build the 8 per-core input maps (host-side sharding + layout prep)."""
    src = np.ascontiguousarray(src, dtype=np.float32)
    tgt = np.ascontiguousarray(tgt, dtype=np.float32)
    src_f = np.ascontiguousarray(src_f, dtype=np.float32)
    tgt_f = np.ascontiguousarray(tgt_f, dtype=np.float32)
    aa = (src_f * src_f).sum(axis=1)  # [B, N]
    bb = (tgt_f * tgt_f).sum(axis=1)  # [B, M]

    def chunk_v(pts):  # [L, 3] -> [MB, VW] fp16, col 4*c+f = [pts|1][c*MB+p, f]
        v = np.concatenate([pts, np.ones((pts.shape[0], 1), np.float32)], axis=1)
        flat = v.reshape(-1, MB, 4).transpose(1, 0, 2).reshape(MB, -1)
        out = np.zeros((MB, VW), np.float32)
        out[:, : flat.shape[1]] = flat
        return np.ascontiguousarray(out.astype(np.float16))

    def side(L, R, bias_m, shift_n, vpts, sl):
        ones_m = np.ones((1, L.shape[1]), np.float16)
        ones_n = np.ones((1, H), np.float16)
        bh, bl = _hi_lo16((C0 - bias_m) * 0.5)
        shift = (-0.5 * shift_n[sl]).astype(np.float16)
        return {
            "lhs": np.ascontiguousarray(
                np.vstack([L.astype(np.float16), ones_m, bh[None, :], bl[None, :]])
            ),
            "rhs": np.ascontiguousarray(
                np.vstack([R[:, sl].astype(np.float16), shift[None, :], ones_n, ones_n])
            ),
            "v": chunk_v(vpts),
        }

    in_maps = []
    for c in range(NCORES):
        b, h = divmod(c, 2)
        sl = slice(h * H, (h + 1) * H)
        A = side(tgt_f[b], src_f[b], bb[b], aa[b], tgt[b], sl)
        Bs = side(src_f[b], tgt_f[b], aa[b], bb[b], src[b], sl)
        m = {k + "A": v for k, v in A.items()}
        m.update({k + "B": v for k, v in Bs.items()})
        in_maps.append(m)
    return in_maps


def run(inputs, trace=False, **kw):
    global _PROG
    if _PROG is None:
        _PROG = _build()
    in_maps = _prep_inputs(
        inputs["src"], inputs["tgt"], inputs["src_f"], inputs["tgt_f"]
    )
    bkr = run_bass_kernel_spmd(
        _PROG, in_maps, core_ids=list(range(NCORES)), trace=trace, **kw
    )
    src_corr = np.zeros((B, N, 3), np.float32)
    tgt_corr = np.zeros((B, M, 3), np.float32)
    for c in range(NCORES):
        b, h = divmod(c, 2)
        sl = slice(h * H, (h + 1) * H)
        ws = bkr.results[c]["out_src"]
        wt = bkr.results[c]["out_tgt"]
        src_corr[b, sl] = (ws[0:3] / ws[3:4]).T
        tgt_corr[b, sl] = (wt[0:3] / wt[3:4]).T
    return (src_corr, tgt_corr), bkr


def kernel(**inputs):
    out, _ = run(inputs)
    return out
